# revision 20
# baseline (speedup 1.0000x reference)
"""Trainium2 Bass kernel: Conv1d(200->512,w3) + tanh + masked avg-pool encodings
+ cosine similarities, data-parallel over the batch dim on 8 NeuronCores.

v4: pad-sparsity aware + host-side im2col.
- Entries with pad==0 contribute exactly 0, so the host packs only
  title-active / body-active sequences (fixed capacity, mean+6.8sigma) and the
  device runs two uniform conv+tanh+sum passes.
- The conv is a K=601 matmul: the host lays x out as 3 tap-shifted copies of
  the 200 channels plus a ones row (which pairs with a conv_b weight row), so
  each PSUM tile needs only 5 accumulating matmuls instead of 6 and no
  device-side shifts.
- Masking, the 0.5 combine, and the tiny cosine tail run on the host.

Self-contained: hardcodes all shapes. kernel(**inputs) takes the full fp32
inputs and returns (matrix [128,21], out [128,22,512]) like the reference.
"""

import os
from contextlib import ExitStack

import ml_dtypes
import numpy as np

import concourse.bass as bass
import concourse.tile as tile
from concourse import bacc, mybir
from concourse.bass_utils import run_bass_kernel_spmd

# Problem shapes (fixed).
B, N, C, L = 128, 22, 200, 125
H, W = 512, 3
LT, LB = 25, 100             # title/body lengths
NCORES = 8
S = B // NCORES              # samples per core (16)
NS = S * N                   # sequences per core (352)
HCH = H // 128               # H chunks (4)
NT, NB = LT - W + 1, LB - W + 1   # 23 title / 98 body conv positions
EPS = 1e-8

KIM = W * C + 1              # im2col contraction size incl. ones row (601)
KCH = [128, 128, 128, 128, KIM - 512]   # K chunks (last: 89)

# Packed-pass geometry: both passes use groups of 500 columns.
GT = 20                      # title seqs per group  (20 * 25  = 500)
GB = 5                       # body  seqs per group  ( 5 * 100 = 500)
GL = 500
CAP_STEP = 20                # capacities rounded up to this (limits recompiles)

F32 = mybir.dt.float32
BF16 = mybir.dt.bfloat16
BF16NP = ml_dtypes.bfloat16

_CACHE = {}

# Set by each kernel() call when tracing is enabled (BASS_KERNEL_TRACE=1).
last_exec_time_ns = None
last_mean_exec_time_ns = None


def _build_nc(cap_t, cap_b):
    ng_t, ng_b = cap_t // GT, cap_b // GB
    nc = bacc.Bacc("TRN2", target_bir_lowering=False, debug=False,
                   num_devices=NCORES)
    xt_in = nc.declare_dram_parameter("xt_in", [KIM, cap_t * LT], BF16,
                                      isOutput=False)
    xb_in = nc.declare_dram_parameter("xb_in", [KIM, cap_b * LB], BF16,
                                      isOutput=False)
    w_in = nc.declare_dram_parameter("w_in", [KIM, H], BF16, isOutput=False)
    st_out = nc.declare_dram_parameter("st_out", [HCH, 128, cap_t], F32,
                                       isOutput=True)
    sb_out = nc.declare_dram_parameter("sb_out", [HCH, 128, cap_b], F32,
                                       isOutput=True)

    with ExitStack() as ctx:
        tc = ctx.enter_context(tile.TileContext(nc))
        const_pool = ctx.enter_context(tc.tile_pool(name="const", bufs=1))
        acc_pool = ctx.enter_context(tc.tile_pool(name="acc", bufs=1))
        z_pool = ctx.enter_context(tc.tile_pool(name="z", bufs=6))

        # Weights in lhsT layout [k, h]; k = w*200 + c, row 600 is conv_b
        # (paired with the constant ones row the host appends to the data).
        wk = []
        r0 = 0
        for j, kj in enumerate(KCH):
            t = const_pool.tile([kj, H], BF16, tag=f"wk{j}")
            nc.scalar.dma_start(t[:], w_in[r0:r0 + kj, :])
            wk.append(t)
            r0 += kj

        st_acc = acc_pool.tile([128, HCH * cap_t], F32)
        sb_acc = acc_pool.tile([128, HCH * cap_b], F32)

        passes = (
            (xt_in, ng_t, st_acc, GT, NT, st_out, cap_t),
            (xb_in, ng_b, sb_acc, GB, NB, sb_out, cap_b),
        )
        with tc.tile_pool(name="psum", bufs=2, space="PSUM") as psum_pool:
            for (x_in, ngroups, acc, gs, npos, s_out, cap) in passes:
                acc_v = acc[:].rearrange("p (h j) -> p h j", h=HCH)
                # K rows 0..511 as 4 column-blocks of one 128-partition view.
                x_hi = x_in[0:512, :].rearrange("(blk p) n -> p blk n", p=128)
                for g in range(ngroups):
                    # Split the group load across the SP and ACT HWDGE queues
                    # plus the (otherwise idle) GpSimd SWDGE to keep pace
                    # with the PE.
                    z0 = z_pool.tile([128, 4 * GL], BF16, tag="z0")
                    z0v = z0[:].rearrange("p (blk n) -> p blk n", blk=4)
                    sl = slice(g * GL, (g + 1) * GL)
                    nc.sync.dma_start(z0v[:, 0:2], x_hi[:, 0:2, sl])
                    # The scalar HWDGE queue serves the weight loads first, so
                    # the opening groups go fully through sync to avoid an
                    # initial PE stall.
                    eng2 = nc.sync if (x_in is xt_in and g < 2) else nc.scalar
                    eng2.dma_start(z0v[:, 2:4], x_hi[:, 2:4, sl])
                    z1 = z_pool.tile([KCH[4], GL], BF16, tag="z1")
                    nc.gpsimd.dma_start(z1[:], x_in[512:KIM, sl])

                    ps = psum_pool.tile([128, HCH * 512], F32)  # 4 PSUM banks
                    for h in range(HCH):
                        for j, kj in enumerate(KCH):
                            rhs = (z0[:, j * GL:(j + 1) * GL] if j < 4
                                   else z1[0:kj, 0:GL])
                            nc.tensor.matmul(
                                out=ps[:, h * 512:h * 512 + GL],
                                lhsT=wk[j][0:kj, h * 128:(h + 1) * 128],
                                rhs=rhs,
                                start=(j == 0),
                                stop=(j == len(KCH) - 1),
                            )

                    # tanh in place on PSUM (bias added via the ones K-row).
                    v = ps[:].rearrange("p (h r) -> p h r", h=HCH)[:, :, 0:GL]
                    nc.scalar.activation(v, v, mybir.ActivationFunctionType.Tanh)

                    # Per-sequence sums over the valid conv positions.
                    v4 = v.rearrange("p h (s l) -> p h s l", s=gs)
                    nc.vector.tensor_reduce(
                        out=acc_v[:, :, g * gs:(g + 1) * gs],
                        in_=v4[:, :, :, 0:npos],
                        axis=mybir.AxisListType.X,
                        op=mybir.AluOpType.add,
                    )

                # Results out on the lightly-loaded GpSimd queue, emitted per
                # pass so the title outputs overlap the body compute.
                for h in range(HCH):
                    nc.gpsimd.dma_start(s_out[h],
                                        acc[:, h * cap:(h + 1) * cap])

    nc.compile()
    return nc


def _pack_im2col(xseqs, idx, cap, slen):
    """xseqs [NS, C, slen] fp32 -> [KIM, cap*slen] bf16: active seqs packed,
    3 tap-shifted channel blocks + ones row."""
    out = np.zeros((KIM, cap * slen), BF16NP)
    k = len(idx)
    if k:
        g = np.ascontiguousarray(xseqs[idx].transpose(1, 0, 2)) \
            .reshape(C, k * slen).astype(BF16NP)
        n = k * slen
        out[0:C, :n] = g
        out[C:2 * C, :n - 1] = g[:, 1:]
        out[2 * C:3 * C, :n - 2] = g[:, 2:]
    out[3 * C, :] = BF16NP(1.0)
    return out


def kernel(x, pad_title, pad_body, conv_w, conv_b):
    global last_exec_time_ns, last_mean_exec_time_ns
    x = np.asarray(x, dtype=np.float32)
    pad_title = np.asarray(pad_title, dtype=np.float32)
    pad_body = np.asarray(pad_body, dtype=np.float32)
    conv_w = np.asarray(conv_w, dtype=np.float32)
    conv_b = np.asarray(conv_b, dtype=np.float32)

    t_idx = [np.nonzero(pad_title[c * S:(c + 1) * S].ravel())[0]
             for c in range(NCORES)]
    b_idx = [np.nonzero(pad_body[c * S:(c + 1) * S].ravel())[0]
             for c in range(NCORES)]

    def _cap(idxs):
        m = max(len(i) for i in idxs)
        return max(CAP_STEP, -(-m // CAP_STEP) * CAP_STEP)

    cap_t, cap_b = _cap(t_idx), _cap(b_idx)
    if (cap_t, cap_b) not in _CACHE:
        _CACHE[(cap_t, cap_b)] = _build_nc(cap_t, cap_b)
    nc = _CACHE[(cap_t, cap_b)]

    # Weights -> lhsT layout [w*200+c, h]; row 600 carries conv_b.
    w600 = np.empty((KIM, H), np.float32)
    w600[:3 * C] = conv_w.transpose(2, 1, 0).reshape(3 * C, H)
    w600[3 * C] = conv_b
    w600 = w600.astype(BF16NP)

    in_maps = []
    for c in range(NCORES):
        xs = x[c * S:(c + 1) * S].reshape(NS, C, L)
        in_maps.append({
            "xt_in": _pack_im2col(xs[:, :, :LT], t_idx[c], cap_t, LT),
            "xb_in": _pack_im2col(xs[:, :, LT:], b_idx[c], cap_b, LB),
            "w_in": w600,
        })

    trace = os.environ.get("BASS_KERNEL_TRACE", "0") == "1"
    res = run_bass_kernel_spmd(nc, in_maps, list(range(NCORES)), trace=trace)
    last_exec_time_ns = res.exec_time_ns
    last_mean_exec_time_ns = res.mean_exec_time_ns

    out = np.empty((B, N, H), np.float32)
    matrix = np.empty((B, N - 1), np.float32)
    for c in range(NCORES):
        st = res.results[c]["st_out"]            # [HCH, 128, cap_t]
        sb = res.results[c]["sb_out"]            # [HCH, 128, cap_b]
        enc = np.zeros((NS, H), np.float32)
        ti, bi = t_idx[c], b_idx[c]
        pt = pad_title[c * S:(c + 1) * S].ravel()
        pb = pad_body[c * S:(c + 1) * S].ravel()
        if len(ti):
            st_seq = st.transpose(2, 0, 1).reshape(cap_t, H)[:len(ti)]
            enc[ti] += (0.5 * pt[ti])[:, None] * st_seq
        if len(bi):
            sb_seq = sb.transpose(2, 0, 1).reshape(cap_b, H)[:len(bi)]
            enc[bi] += (0.5 * pb[bi])[:, None] * sb_seq
        enc = enc.reshape(S, N, H)
        out[c * S:(c + 1) * S] = enc
        num = np.sum(enc[:, 1:] * enc[:, :1], axis=-1)
        den = np.maximum(
            np.linalg.norm(enc[:, 1:], axis=-1) * np.linalg.norm(enc[:, :1], axis=-1),
            EPS)
        matrix[c * S:(c + 1) * S] = num / den
    return matrix, out


def _host_fallback(x, pad_title, pad_body, conv_w, conv_b):
    """Exact fp32 numpy path, used only if a pack capacity would overflow."""
    z = x.reshape(B * N, C, L)

    def encode(seg):
        l = seg.shape[-1]
        y = np.zeros((B * N, H, l - W + 1), np.float32)
        for w in range(W):
            y += np.einsum("hc,scj->shj", conv_w[:, :, w],
                           seg[:, :, w:w + l - W + 1], optimize=True)
        y = np.tanh(y + conv_b[None, :, None])
        return y.mean(axis=-1).reshape(B, N, H)

    enc_t = encode(z[:, :, :LT]) * (LT - W + 1) * pad_title[..., None]
    enc_b = encode(z[:, :, LT:]) * (LB - W + 1) * pad_body[..., None]
    out = 0.5 * (enc_t + enc_b)
    main, Q = out[:, :1, :], out[:, 1:, :]
    num = np.sum(Q * main, axis=-1)
    den = np.maximum(
        np.linalg.norm(Q, axis=-1) * np.linalg.norm(main, axis=-1), EPS)
    return num / den, out


# revision 22
# speedup vs baseline: 1.0875x; 1.0875x over previous
"""Trainium2 Bass kernel: Conv1d(200->512,w3) + tanh + masked avg-pool encodings
+ cosine similarities, data-parallel over the batch dim on 8 NeuronCores.

v4: pad-sparsity aware + host-side im2col.
- Entries with pad==0 contribute exactly 0, so the host packs only
  title-active / body-active sequences (fixed capacity, mean+6.8sigma) and the
  device runs two uniform conv+tanh+sum passes.
- The conv is a K=601 matmul: the host lays x out as 3 tap-shifted copies of
  the 200 channels plus a ones row (which pairs with a conv_b weight row), so
  each PSUM tile needs only 5 accumulating matmuls instead of 6 and no
  device-side shifts.
- Masking, the 0.5 combine, and the tiny cosine tail run on the host.

Self-contained: hardcodes all shapes. kernel(**inputs) takes the full fp32
inputs and returns (matrix [128,21], out [128,22,512]) like the reference.
"""

import os
from contextlib import ExitStack

import ml_dtypes
import numpy as np

import concourse.bass as bass
import concourse.tile as tile
from concourse import bacc, mybir
from concourse.bass_utils import run_bass_kernel_spmd

# Problem shapes (fixed).
B, N, C, L = 128, 22, 200, 125
H, W = 512, 3
LT, LB = 25, 100             # title/body lengths
NCORES = 8
S = B // NCORES              # samples per core (16)
NS = S * N                   # sequences per core (352)
HCH = H // 128               # H chunks (4)
NT, NB = LT - W + 1, LB - W + 1   # 23 title / 98 body conv positions
EPS = 1e-8

KIM = W * C + 1              # im2col contraction size incl. ones row (601)
KCH = [128, 128, 128, 128, KIM - 512]   # K chunks (last: 89)

# Packed-pass geometry: both passes use groups of 500 columns.
GT = 20                      # title seqs per group  (20 * 25  = 500)
GB = 5                       # body  seqs per group  ( 5 * 100 = 500)
GL = 500
CAP_STEP = 20                # capacities rounded up to this (limits recompiles)

F32 = mybir.dt.float32
BF16 = mybir.dt.bfloat16
BF16NP = ml_dtypes.bfloat16

_CACHE = {}

# Set by each kernel() call when tracing is enabled (BASS_KERNEL_TRACE=1).
last_exec_time_ns = None
last_mean_exec_time_ns = None


def _build_nc(cap_t, cap_b):
    ng_t, ng_b = cap_t // GT, cap_b // GB
    nc = bacc.Bacc("TRN2", target_bir_lowering=False, debug=False,
                   num_devices=NCORES)
    xt_in = nc.declare_dram_parameter("xt_in", [KIM, cap_t * LT], BF16,
                                      isOutput=False)
    xb_in = nc.declare_dram_parameter("xb_in", [KIM, cap_b * LB], BF16,
                                      isOutput=False)
    w_in = nc.declare_dram_parameter("w_in", [KIM, H], BF16, isOutput=False)
    st_out = nc.declare_dram_parameter("st_out", [HCH, 128, cap_t], F32,
                                       isOutput=True)
    sb_out = nc.declare_dram_parameter("sb_out", [HCH, 128, cap_b], F32,
                                       isOutput=True)

    with ExitStack() as ctx:
        tc = ctx.enter_context(tile.TileContext(nc))
        const_pool = ctx.enter_context(tc.tile_pool(name="const", bufs=1))
        acc_pool = ctx.enter_context(tc.tile_pool(name="acc", bufs=1))
        z_pool = ctx.enter_context(tc.tile_pool(name="z", bufs=6))

        # Weights in lhsT layout [k, h]; k = w*200 + c, row 600 is conv_b
        # (paired with the constant ones row the host appends to the data).
        wk = []
        r0 = 0
        for j, kj in enumerate(KCH):
            t = const_pool.tile([kj, H], BF16, tag=f"wk{j}")
            nc.scalar.dma_start(t[:], w_in[r0:r0 + kj, :])
            wk.append(t)
            r0 += kj

        st_acc = acc_pool.tile([128, HCH * cap_t], F32)
        sb_acc = acc_pool.tile([128, HCH * cap_b], F32)

        passes = (
            (xt_in, ng_t, st_acc, GT, NT, st_out, cap_t),
            (xb_in, ng_b, sb_acc, GB, NB, sb_out, cap_b),
        )
        with tc.tile_pool(name="psum", bufs=4, space="PSUM") as psum_pool:
            for (x_in, ngroups, acc, gs, npos, s_out, cap) in passes:
                acc_v = acc[:].rearrange("p (h j) -> p h j", h=HCH)
                # K rows 0..511 as 4 column-blocks of one 128-partition view.
                x_hi = x_in[0:512, :].rearrange("(blk p) n -> p blk n", p=128)
                for g in range(ngroups):
                    # Split the group load across the SP and ACT HWDGE queues
                    # plus the (otherwise idle) GpSimd SWDGE to keep pace with
                    # the PE. The opening groups go fully through sync (the
                    # scalar queue serves the weight loads first, and the
                    # SWDGE is slow to start), ordered so the K-chunks the PE
                    # needs first land first.
                    early = x_in is xt_in and g < 3
                    z0 = z_pool.tile([128, 4 * GL], BF16, tag="z0")
                    z0v = z0[:].rearrange("p (blk n) -> p blk n", blk=4)
                    sl = slice(g * GL, (g + 1) * GL)
                    z1 = z_pool.tile([KCH[4], GL], BF16, tag="z1")
                    nc.sync.dma_start(z0v[:, 0:2], x_hi[:, 0:2, sl])
                    if early:
                        nc.sync.dma_start(z1[:], x_in[512:KIM, sl])
                        nc.sync.dma_start(z0v[:, 2:4], x_hi[:, 2:4, sl])
                    else:
                        nc.scalar.dma_start(z0v[:, 2:4], x_hi[:, 2:4, sl])
                        nc.gpsimd.dma_start(z1[:], x_in[512:KIM, sl])

                    for hh in range(HCH // 2):  # two 2-bank PSUM tiles/group
                        ps = psum_pool.tile([128, 2 * 512], F32)
                        for hl in range(2):
                            h = hh * 2 + hl
                            for j, kj in enumerate(KCH):
                                rhs = (z0[:, j * GL:(j + 1) * GL] if j < 4
                                       else z1[0:kj, 0:GL])
                                nc.tensor.matmul(
                                    out=ps[:, hl * 512:hl * 512 + GL],
                                    lhsT=wk[j][0:kj, h * 128:(h + 1) * 128],
                                    rhs=rhs,
                                    start=(j == 0),
                                    stop=(j == len(KCH) - 1),
                                )

                        # tanh in place on PSUM (bias added via ones K-row).
                        v = ps[:].rearrange("p (h r) -> p h r", h=2)[:, :, 0:GL]
                        nc.scalar.activation(
                            v, v, mybir.ActivationFunctionType.Tanh)

                        # Per-sequence sums over the valid conv positions.
                        v4 = v.rearrange("p h (s l) -> p h s l", s=gs)
                        nc.vector.tensor_reduce(
                            out=acc_v[:, hh * 2:(hh + 1) * 2,
                                      g * gs:(g + 1) * gs],
                            in_=v4[:, :, :, 0:npos],
                            axis=mybir.AxisListType.X,
                            op=mybir.AluOpType.add,
                        )

                # Results out on the lightly-loaded GpSimd queue, emitted per
                # pass so the title outputs overlap the body compute.
                for h in range(HCH):
                    nc.gpsimd.dma_start(s_out[h],
                                        acc[:, h * cap:(h + 1) * cap])

    nc.compile()
    return nc


def _pack_im2col(xseqs, idx, cap, slen):
    """xseqs [NS, C, slen] fp32 -> [KIM, cap*slen] bf16: active seqs packed,
    3 tap-shifted channel blocks + ones row."""
    out = np.zeros((KIM, cap * slen), BF16NP)
    k = len(idx)
    if k:
        g = np.ascontiguousarray(xseqs[idx].transpose(1, 0, 2)) \
            .reshape(C, k * slen).astype(BF16NP)
        n = k * slen
        out[0:C, :n] = g
        out[C:2 * C, :n - 1] = g[:, 1:]
        out[2 * C:3 * C, :n - 2] = g[:, 2:]
    out[3 * C, :] = BF16NP(1.0)
    return out


def kernel(x, pad_title, pad_body, conv_w, conv_b):
    global last_exec_time_ns, last_mean_exec_time_ns
    x = np.asarray(x, dtype=np.float32)
    pad_title = np.asarray(pad_title, dtype=np.float32)
    pad_body = np.asarray(pad_body, dtype=np.float32)
    conv_w = np.asarray(conv_w, dtype=np.float32)
    conv_b = np.asarray(conv_b, dtype=np.float32)

    t_idx = [np.nonzero(pad_title[c * S:(c + 1) * S].ravel())[0]
             for c in range(NCORES)]
    b_idx = [np.nonzero(pad_body[c * S:(c + 1) * S].ravel())[0]
             for c in range(NCORES)]

    def _cap(idxs):
        m = max(len(i) for i in idxs)
        return max(CAP_STEP, -(-m // CAP_STEP) * CAP_STEP)

    cap_t, cap_b = _cap(t_idx), _cap(b_idx)
    if (cap_t, cap_b) not in _CACHE:
        _CACHE[(cap_t, cap_b)] = _build_nc(cap_t, cap_b)
    nc = _CACHE[(cap_t, cap_b)]

    # Weights -> lhsT layout [w*200+c, h]; row 600 carries conv_b.
    w600 = np.empty((KIM, H), np.float32)
    w600[:3 * C] = conv_w.transpose(2, 1, 0).reshape(3 * C, H)
    w600[3 * C] = conv_b
    w600 = w600.astype(BF16NP)

    in_maps = []
    for c in range(NCORES):
        xs = x[c * S:(c + 1) * S].reshape(NS, C, L)
        in_maps.append({
            "xt_in": _pack_im2col(xs[:, :, :LT], t_idx[c], cap_t, LT),
            "xb_in": _pack_im2col(xs[:, :, LT:], b_idx[c], cap_b, LB),
            "w_in": w600,
        })

    trace = os.environ.get("BASS_KERNEL_TRACE", "0") == "1"
    res = run_bass_kernel_spmd(nc, in_maps, list(range(NCORES)), trace=trace)
    last_exec_time_ns = res.exec_time_ns
    last_mean_exec_time_ns = res.mean_exec_time_ns

    out = np.empty((B, N, H), np.float32)
    matrix = np.empty((B, N - 1), np.float32)
    for c in range(NCORES):
        st = res.results[c]["st_out"]            # [HCH, 128, cap_t]
        sb = res.results[c]["sb_out"]            # [HCH, 128, cap_b]
        enc = np.zeros((NS, H), np.float32)
        ti, bi = t_idx[c], b_idx[c]
        pt = pad_title[c * S:(c + 1) * S].ravel()
        pb = pad_body[c * S:(c + 1) * S].ravel()
        if len(ti):
            st_seq = st.transpose(2, 0, 1).reshape(cap_t, H)[:len(ti)]
            enc[ti] += (0.5 * pt[ti])[:, None] * st_seq
        if len(bi):
            sb_seq = sb.transpose(2, 0, 1).reshape(cap_b, H)[:len(bi)]
            enc[bi] += (0.5 * pb[bi])[:, None] * sb_seq
        enc = enc.reshape(S, N, H)
        out[c * S:(c + 1) * S] = enc
        num = np.sum(enc[:, 1:] * enc[:, :1], axis=-1)
        den = np.maximum(
            np.linalg.norm(enc[:, 1:], axis=-1) * np.linalg.norm(enc[:, :1], axis=-1),
            EPS)
        matrix[c * S:(c + 1) * S] = num / den
    return matrix, out


def _host_fallback(x, pad_title, pad_body, conv_w, conv_b):
    """Exact fp32 numpy path, used only if a pack capacity would overflow."""
    z = x.reshape(B * N, C, L)

    def encode(seg):
        l = seg.shape[-1]
        y = np.zeros((B * N, H, l - W + 1), np.float32)
        for w in range(W):
            y += np.einsum("hc,scj->shj", conv_w[:, :, w],
                           seg[:, :, w:w + l - W + 1], optimize=True)
        y = np.tanh(y + conv_b[None, :, None])
        return y.mean(axis=-1).reshape(B, N, H)

    enc_t = encode(z[:, :, :LT]) * (LT - W + 1) * pad_title[..., None]
    enc_b = encode(z[:, :, LT:]) * (LB - W + 1) * pad_body[..., None]
    out = 0.5 * (enc_t + enc_b)
    main, Q = out[:, :1, :], out[:, 1:, :]
    num = np.sum(Q * main, axis=-1)
    den = np.maximum(
        np.linalg.norm(Q, axis=-1) * np.linalg.norm(main, axis=-1), EPS)
    return num / den, out
